# revision 47
# baseline (speedup 1.0000x reference)
import math
from contextlib import ExitStack

import numpy as np

N, T, D, H = 512, 128, 512, 512
NC = 8
n = N // NC          # 64 samples per core
H4 = 4 * H           # 2048
SCALE = 1.0 / math.sqrt(H)
DEBUG = False
PF = 3               # xw prefetch ring depth

_cache = {}

# column offsets of h-chunk k inside hT_sb [128, 256]
# (hT_sb cols 0:128 = transpose of h2b[:,0:128] -> (kh, s) = chunks 0, 2;
#  cols 128:256 = transpose of h2b[:,128:256] -> chunks 1, 3)
KCOL = {0: 0, 2: 64, 1: 128, 3: 192}


def _build_kernel():
    if "nc" in _cache:
        return _cache["nc"]

    import concourse.bass as bass
    import concourse.tile as tile
    from concourse import bacc, mybir

    f32 = mybir.dt.float32
    bf16 = mybir.dt.bfloat16
    ALU = mybir.AluOpType
    ACTF = mybir.ActivationFunctionType
    AX = mybir.AxisListType

    nc = bacc.Bacc(
        "TRN2",
        target_bir_lowering=False,
        debug=False,
        enable_asserts=False,
        num_devices=NC,
    )

    # ---- DRAM tensors ----------------------------------------------------
    xT = nc.dram_tensor("xT", (D, n * T), bf16, kind="ExternalInput").ap()
    AhT = nc.dram_tensor("AhT", (128, 4 * 8 * 128), bf16, kind="ExternalInput").ap()
    Asp = nc.dram_tensor("Asp", (128, 8 * 4 * 128), bf16, kind="ExternalInput").ap()
    Wd = nc.dram_tensor("Wd", (128, 8 * 4 * 2 * 256), bf16, kind="ExternalInput").ap()
    Wxd = nc.dram_tensor("Wxd", (128, 4 * 4 * 512), bf16, kind="ExternalInput").ap()
    h02d = nc.dram_tensor("h02", (128, 256), f32, kind="ExternalInput").ap()
    h0Td = nc.dram_tensor("h0T", (128, 256), bf16, kind="ExternalInput").ap()
    bd = nc.dram_tensor("bd", (1, H4), bf16, kind="ExternalInput").ap()
    G128d = nc.dram_tensor("G128", (128, 128), bf16, kind="ExternalInput").ap()
    mQ8d = nc.dram_tensor("mQ8", (128, 8), bf16, kind="ExternalInput").ap()
    id128d = nc.dram_tensor("id128", (128, 128), bf16, kind="ExternalInput").ap()
    id65d = nc.dram_tensor("id65", (65, 64), bf16, kind="ExternalInput").ap()
    hs = nc.dram_tensor("hs", (T, 2, 64, 256), f32, kind="ExternalOutput").ap()
    xwd = nc.dram_tensor("xwd", (T // 2, 2, n, H4), bf16, kind="Internal").ap()
    dbg = None
    if DEBUG:
        dbg = {nm: nc.dram_tensor(f"dbg_{nm}", shp, dt, kind="ExternalOutput").ap()
               for nm, shp, dt in [
                   ("dot2", (128, 8), f32),
                   ("w128", (128, 64), f32),
                   ("att", (128, 256), f32),
                   ("hb", (128, 512), f32),
                   ("xw0", (128, 512), f32),
               ]}

    with tile.TileContext(nc) as tc, ExitStack() as ctx:
        const = ctx.enter_context(tc.tile_pool(name="const", bufs=1))

        # ---- persistent SBUF tiles --------------------------------------
        Wsb = const.tile([128, 8 * 4 * 2 * 256], bf16)
        Wxsb = const.tile([128, 4 * 4 * 512], bf16)
        AhT_sb = const.tile([128, 4 * 8 * 128], bf16)
        Asp_sb = const.tile([128, 8 * 4 * 128], bf16)
        G128 = const.tile([128, 128], bf16)
        mQ8 = const.tile([128, 8], bf16)
        id128 = const.tile([128, 128], bf16)
        id65 = const.tile([65, 64], bf16)
        h2f = const.tile([128, 256], f32)     # h (f32, for hs output)
        hT0 = const.tile([128, 256], bf16)    # initial h transposed
        c_st = const.tile([128, 256], f32)    # c state
        xw_ring = [const.tile([65, H4], bf16, name=f"xw_ring{i}")
                   for i in range(PF)]

        nc.sync.dma_start(Wxsb[:], Wxd[:])
        nc.scalar.dma_start(Wsb[:], Wd[:])
        nc.scalar.dma_start(AhT_sb[:], AhT[:])
        nc.scalar.dma_start(Asp_sb[:], Asp[:])
        nc.scalar.dma_start(G128[:], G128d[:])
        nc.scalar.dma_start(mQ8[:], mQ8d[:])
        nc.scalar.dma_start(id128[:], id128d[:])
        nc.scalar.dma_start(id65[:], id65d[:])
        nc.scalar.dma_start(hT0[:], h0Td[:])
        nc.scalar.dma_start(c_st[:], h02d[:])
        for i in range(PF):
            nc.scalar.dma_start(xw_ring[i][64:65, :], bd[:])

        # xT is host-prepped TIME-major: col = t*n + s
        xT_r = xT.rearrange("(dk dp) (tp c2) -> tp dp dk c2",
                            dp=128, c2=2 * n)
        xw_r = xwd.rearrange("c u s k -> (c u) s k")

        # ---- recurrence with streamed x@Wx precompute -------------------
        # time-pair chunks emitted just ahead of consumption fill the PE
        # bubbles left by the serial h-chain; 2-bank psum ping-pong keeps
        # the pc matmuls from stalling the PE stream on evacuations.
        with tc.tile_pool(name="work", bufs=2) as work, \
             tc.tile_pool(name="pc_x", bufs=3) as xpool, \
             tc.tile_pool(name="pc_o", bufs=3) as opool, \
             tc.tile_pool(name="ps_m", bufs=1, space="PSUM") as ps_m, \
             tc.tile_pool(name="pc_ps", bufs=2, space="PSUM") as pcps, \
             tc.tile_pool(name="ps_g", bufs=1, space="PSUM") as ps_g:

            def emit_pc_pair(c):
                xt = xpool.tile([128, 4 * 2 * 64], bf16)  # [dp, (dk, tt, s)]
                nc.sync.dma_start(
                    xt[:].rearrange("p (dk c2) -> p dk c2", dk=4),
                    xT_r[c])
                ow = opool.tile([128, H4], bf16)
                for q in range(4):
                    pxw = pcps.tile([128, 512], f32, name="pxw")
                    for dk in range(4):
                        nc.tensor.matmul(
                            pxw[:],
                            xt[:, dk * 128:(dk + 1) * 128],
                            Wxsb[:, (dk * 4 + q) * 512:(dk * 4 + q + 1) * 512],
                            start=(dk == 0),
                            stop=(dk == 3),
                        )
                    nc.vector.tensor_copy(ow[:, q * 512:(q + 1) * 512],
                                          pxw[:])
                nc.sync.dma_start(
                    xwd[c].rearrange("u s k -> (u s) k"), ow[:])

            for c0 in range(4):
                emit_pc_pair(c0)

            # prefetch xw for first PF steps
            for t0 in range(PF):
                nc.sync.dma_start(xw_ring[t0][0:64, :], xw_r[t0])

            for t in range(T):
                if t % 2 == 0 and 4 + t // 2 < T // 2:
                    emit_pc_pair(4 + t // 2)

                # -- GEMM scaffolding; xw inject first (h-independent) ------
                gIF = ps_g.tile([128, 512], f32, bufs=2)
                gOG = ps_g.tile([128, 512], f32, bufs=2)
                gtile = {0: (gIF, 0), 1: (gIF, 256), 2: (gOG, 0), 3: (gOG, 256)}
                xw = xw_ring[t % PF]
                attnT = work.tile([128, 256], bf16)

                # psum group per (bank, col-group partitions): start on the
                # xw inject (first touch, h-independent -> fills the PE
                # during the previous step's gate chain), stop on the last
                # attn matmul (q=1/3, j=7)
                def gemm_inject():
                    for q in range(4):
                        gt, co = gtile[q]
                        for m in range(2):
                            nc.tensor.matmul(
                                gt[m * 64:(m + 1) * 64, co:co + 256], id65[:],
                                xw[:, q * 512 + m * 256:q * 512 + (m + 1) * 256],
                                start=(q in (0, 2)),
                                stop=False,
                                tile_position=(0, m * 64),
                            )

                def gemm_part(j_range, q_order=(0, 1, 2, 3), stop_q=()):
                    for q in q_order:
                        gt, co = gtile[q]
                        for m in range(2):
                            out = gt[m * 64:(m + 1) * 64, co:co + 256]
                            tp = (0, m * 64)
                            for j in j_range:
                                if j < 4:
                                    lhsT = hT[:, KCOL[j]:KCOL[j] + 64]
                                else:
                                    lhsT = attnT[:, (j - 4) * 64:(j - 3) * 64]
                                nc.tensor.matmul(
                                    out, lhsT,
                                    Wsb[:, ((j * 4 + q) * 2 + m) * 256:
                                           ((j * 4 + q) * 2 + m + 1) * 256],
                                    start=False,
                                    stop=(j == 7 and q in stop_q),
                                    tile_position=tp,
                                )

                gemm_inject()

                # -- hT: h transposed. h = so*tanh(c); transpose so and tc
                # separately (both land earlier than h itself would) and
                # multiply the transposes on DVE.
                if t == 0:
                    hT = hT0
                else:
                    pT = ps_m.tile([128, 512], bf16)
                    nc.tensor.matmul(pT[:, 0:128], p_tc[:, 0:128], id128[:],
                                     is_transpose=True, start=True, stop=False)
                    nc.tensor.matmul(pT[:, 128:256], p_tc[:, 128:256], id128[:],
                                     is_transpose=True, start=False, stop=False)
                    tcT = work.tile([128, 256], bf16)
                    nc.vector.tensor_copy(tcT[:], pT[:, 0:256])
                    nc.tensor.matmul(pT[:, 256:384], p_so[:, 0:128], id128[:],
                                     is_transpose=True, start=False, stop=False)
                    nc.tensor.matmul(pT[:, 384:512], p_so[:, 128:256], id128[:],
                                     is_transpose=True, start=False, stop=True)
                    hT = work.tile([128, 256], bf16)
                    nc.vector.tensor_tensor(hT[:, 0:128], tcT[:, 0:128],
                                            pT[:, 256:384], ALU.mult)
                    nc.vector.tensor_tensor(hT[:, 128:256], tcT[:, 128:256],
                                            pT[:, 384:512], ALU.mult)

                # -- block-diagonal dot matmuls -----------------------------
                # smf bank: cols 0:64 = rt (block dots), 64:72 = Z
                # one psum group: start on first matmul, stop on the Z matmul
                misc = ps_m.tile([128, 512], f32)
                smf = misc[:, 0:128]
                for c in range(8):
                    for k in (0, 2, 1, 3):
                        nc.tensor.matmul(
                            smf[:, c * 8:(c + 1) * 8],
                            AhT_sb[:, (k * 8 + c) * 128:(k * 8 + c + 1) * 128],
                            hT[:, KCOL[k] + c * 8:KCOL[k] + (c + 1) * 8],
                            start=(c == 0 and k == 0),
                            stop=False,
                        )

                # first half of the GEMM h part fills PE during extract/exp
                # (chunks 0,2 read hT cols 0:128 which evacuate first)
                gemm_part([0, 2])

                # -- extract dot2[(s8,p), c] --------------------------------
                pr = work.tile([128, 64], f32)
                nc.vector.tensor_tensor(
                    pr[:].rearrange("q (c s) -> q c s", c=8),
                    smf[:, 0:64].rearrange("q (c s) -> q c s", c=8),
                    mQ8[:].rearrange("q (r s) -> q r s", r=1)
                        .broadcast_to([128, 8, 8]),
                    ALU.mult)
                dot2 = work.tile([128, 8], f32)
                nc.vector.tensor_reduce(
                    dot2[:], pr[:].rearrange("q (c s) -> q c s", c=8),
                    axis=AX.X, op=ALU.add)
                if DEBUG and t == 0:
                    nc.sync.dma_start(dbg["dot2"], dot2[:])

                # -- softmax (sp-major, normalization into W128) ------------
                wexp2 = work.tile([128, 8], bf16)
                nc.scalar.activation(wexp2[:], dot2[:], ACTF.Exp, scale=SCALE)
                nc.tensor.matmul(smf[:, 64:72], G128[:], wexp2[:],
                                 start=False, stop=True)
                # second half of the GEMM h part: overlaps the zinv/wn2/W128
                # DVE chain so the attnT matmuls are never queue-blocked
                gemm_part([1, 3])
                zinv = work.tile([128, 8], f32)
                nc.vector.reciprocal(zinv[:], smf[:, 64:72])
                wn2 = work.tile([128, 8], bf16)
                nc.vector.tensor_tensor(wn2[:], wexp2[:], zinv[:], ALU.mult)
                W128 = work.tile([128, 64], bf16)
                nc.vector.tensor_tensor(
                    W128[:].rearrange("q (c s) -> q c s", c=8),
                    wn2[:].rearrange("q (c r) -> q c r", r=1)
                        .broadcast_to([128, 8, 8]),
                    mQ8[:].rearrange("q (r s) -> q r s", r=1)
                        .broadcast_to([128, 8, 8]),
                    ALU.mult)
                if DEBUG and t == 0:
                    wd_ = work.tile([128, 64], f32)
                    nc.vector.tensor_copy(wd_[:], W128[:])
                    nc.sync.dma_start(dbg["w128"], wd_[:])



                # -- attnT matmuls (normalized, transposed output) ----------
                at = misc[:, 128:384]
                for c in range(8):
                    for k in range(4):
                        nc.tensor.matmul(
                            at[:, k * 64 + c * 8:k * 64 + (c + 1) * 8],
                            Asp_sb[:, (c * 4 + k) * 128:(c * 4 + k + 1) * 128],
                            W128[:, c * 8:(c + 1) * 8],
                            start=(c == 0 and k == 0),
                            stop=(c == 7 and k == 3),
                        )
                nc.vector.tensor_copy(attnT[:, 0:128], at[:, 0:128])
                nc.vector.tensor_copy(attnT[:, 128:256], at[:, 128:256])
                if DEBUG and t == 0:
                    ad_ = work.tile([128, 256], f32)
                    nc.vector.tensor_copy(ad_[:], at[:])
                    nc.sync.dma_start(dbg["att"], ad_[:])

                # attn part, f/g quarters first: their gate ACTs (and the
                # c-state chain) overlap the i/o quarters' matmuls
                gemm_part([4, 5, 6, 7], q_order=(1, 0, 3, 2), stop_q=(0, 2))

                if DEBUG and t == 0:
                    hd_ = work.tile([128, 512], f32)
                    nc.vector.tensor_copy(hd_[:], gIF[:])
                    nc.sync.dma_start(dbg["hb"], hd_[:])

                # -- gates (sigmoid via tanh; single act table) -------------
                t_f = work.tile([128, 256], bf16)
                t_g = work.tile([128, 256], bf16)
                t_i = work.tile([128, 256], bf16)
                t_o = work.tile([128, 256], bf16)
                nc.scalar.activation(t_f[:], gIF[:, 256:512], ACTF.Tanh, scale=0.5)
                nc.scalar.activation(t_i[:], gIF[:, 0:256], ACTF.Tanh, scale=0.5)
                nc.scalar.activation(t_g[:], gOG[:, 256:512], ACTF.Tanh)
                nc.scalar.activation(t_o[:], gOG[:, 0:256], ACTF.Tanh, scale=0.5)

                sf = work.tile([128, 256], bf16)
                si = work.tile([128, 256], bf16)
                so = work.tile([128, 256], bf16)
                fa = work.tile([128, 256], f32)
                ib = work.tile([128, 256], bf16)
                tc_t = work.tile([128, 256], bf16)
                for u in (slice(0, 128), slice(128, 256)):
                    nc.vector.tensor_scalar(sf[:, u], t_f[:, u], 0.5, 0.5,
                                            ALU.mult, ALU.add)
                    nc.vector.tensor_tensor(fa[:, u], sf[:, u], c_st[:, u],
                                            ALU.mult)
                    nc.vector.tensor_scalar(si[:, u], t_i[:, u], 0.5, 0.5,
                                            ALU.mult, ALU.add)
                    nc.vector.tensor_tensor(ib[:, u], si[:, u], t_g[:, u],
                                            ALU.mult)
                    nc.vector.tensor_tensor(c_st[:, u], fa[:, u], ib[:, u],
                                            ALU.add)
                    nc.scalar.activation(tc_t[:, u], c_st[:, u], ACTF.Tanh)
                nc.gpsimd.tensor_scalar(so[:], t_o[:], 0.5, 0.5,
                                         ALU.mult, ALU.add)
                nc.vector.tensor_tensor(h2f[:], so[:], tc_t[:], ALU.mult)
                p_so, p_tc = so, tc_t

                # -- DMA out h, prefetch next xw ----------------------------
                if t + PF < T:
                    nc.sync.dma_start(xw_ring[(t + PF) % PF][0:64, :],
                                      xw_r[t + PF])
                nc.gpsimd.dma_start(hs[t], h2f[:])

    nc.compile()
    _cache["nc"] = nc
    return nc


def _host_prep(x, A, Wx, Wh, Wattn, b):
    import ml_dtypes
    bft = ml_dtypes.bfloat16

    Wcat = np.concatenate([np.asarray(Wh), np.asarray(Wattn)], axis=0)  # (1024, 2048)
    W_host = np.ascontiguousarray(
        Wcat.reshape(8, 128, 4, 2, 256).transpose(1, 0, 2, 3, 4)
        .reshape(128, 8 * 4 * 2 * 256)).astype(bft)
    Wx_host = np.ascontiguousarray(
        np.asarray(Wx).reshape(4, 128, 4, 512).transpose(1, 0, 2, 3)
        .reshape(128, 4 * 4 * 512)).astype(bft)
    b_host = np.asarray(b, dtype=np.float32).reshape(1, H4).astype(bft)

    q = np.arange(128)
    mQ8 = (q[:, None] // 16 == np.arange(8)[None, :]).astype(bft)
    G128 = (q[:, None] // 16 == q[None, :] // 16).astype(bft)
    id128 = np.eye(128, dtype=bft)
    id65 = np.concatenate([np.eye(64, dtype=np.float32),
                           np.ones((1, 64), dtype=np.float32)], axis=0).astype(bft)

    per_core = []
    for k in range(NC):
        xc = np.asarray(x[n * k:n * (k + 1)], dtype=np.float32)   # (64, T, D)
        Ac = np.asarray(A[n * k:n * (k + 1)], dtype=np.float32)   # (64, H, 4, 4)
        xT_host = np.ascontiguousarray(
            xc.transpose(2, 1, 0).reshape(D, n * T)).astype(bft)
        Af = Ac.reshape(n, H, 16).transpose(0, 2, 1)              # (64, 16, 512)
        Af2 = Af.reshape(8, 8, 16, 4, 128)                        # [c, s8, p, k, hp]
        AhT_host = np.ascontiguousarray(
            Af2.transpose(4, 3, 0, 1, 2).reshape(128, 4 * 8 * 128)).astype(bft)
        Asp_host = np.ascontiguousarray(
            Af2.transpose(1, 2, 0, 3, 4).reshape(128, 8 * 4 * 128)).astype(bft)
        h0c = Ac.mean(axis=(2, 3))                                # (64, 512)
        h02 = np.ascontiguousarray(
            h0c.reshape(64, 2, 256).transpose(1, 0, 2).reshape(128, 256)
        ).astype(np.float32)
        hh = h0c.reshape(64, 4, 128)
        h0T = np.ascontiguousarray(np.concatenate(
            [hh[:, k, :].T for k in (0, 2, 1, 3)], axis=1)).astype(bft)
        per_core.append({
            "xT": xT_host, "AhT": AhT_host, "Asp": Asp_host,
            "Wd": W_host, "Wxd": Wx_host, "h02": h02, "h0T": h0T,
            "bd": b_host,
            "G128": G128, "mQ8": mQ8, "id128": id128, "id65": id65,
        })
    return per_core


TRACE = False
LAST_RES = None


def kernel(x, A, Wx, Wh, Wattn, b):
    from concourse import bass_utils

    nc = _build_kernel()
    in_maps = _host_prep(x, A, Wx, Wh, Wattn, b)

    global LAST_RES
    res = bass_utils.run_bass_kernel_spmd(nc, in_maps, core_ids=list(range(NC)),
                                          trace=TRACE)
    LAST_RES = res

    out = np.empty((N, T, H), dtype=np.float32)
    for k in range(NC):
        hs_k = np.asarray(res.results[k]["hs"])           # (T, 2, 64, 256)
        out[n * k:n * (k + 1)] = (
            hs_k.transpose(2, 0, 1, 3).reshape(n, T, H))
    return out
